# revision 25
# baseline (speedup 1.0000x reference)
"""Trainium2 Bass kernel for nn_A100SimilarityCorrector.

Full inputs in, full output out. Data-parallel over the batch: 8 batch
elements -> 8 NeuronCores, one [512,512] similarity slice per core.

Key structure: in eval mode the 4-layer MLP acts independently on each
scalar similarity x, so it collapses to a 1-D function f(x) = sigmoid(g(x))
with g piecewise-linear. BatchNorms fold exactly into the weights; g is
fit on the host with a low-degree polynomial p (exact linear for the
default zero-bias weights), and the device evaluates
    out = (1-I) o (Z + Z^T),  Z = 0.5*rw*(m m^T) o sigmoid(p(A)) + 0.5*(1-rw)*A
with ScalarE (sigmoid; the row mask rides the per-partition bias as a
-50 logit offset), VectorE (column mask / residual / symmetrize adds),
TensorE (16 x 128x128 block transposes into PSUM) and DMA in/out.
"""
import dataclasses
import sys

sys.path.insert(0, "/opt/trn_rl_repo")

import numpy as np
import ml_dtypes

EPS = 1e-5
B, N, P = 8, 512, 128
NT = N // P  # 4 row tiles per core
BIG = 50.0   # logit offset that drives sigmoid to 0 for masked rows


def _fit_scalar_fn(w1, b1, W2, b2, g1, be1, m1, v1, g2, be2, m2, v2,
                   W3, b3, W4, b4, xlo, xhi):
    """Fold BN into weights, then least-squares fit poly p with
    sigmoid(p(x)) ~ f(x) on [xlo, xhi]. Returns (coeffs lowest-first, max_err)."""
    f64 = np.float64
    w1 = w1.astype(f64); b1 = b1.astype(f64)
    s1 = g1.astype(f64) / np.sqrt(v1.astype(f64) + EPS)
    t1 = be1.astype(f64) - m1.astype(f64) * s1
    W2p = s1[:, None] * W2.astype(f64)
    b2p = b2.astype(f64) + t1 @ W2.astype(f64)
    s2 = g2.astype(f64) / np.sqrt(v2.astype(f64) + EPS)
    t2 = be2.astype(f64) - m2.astype(f64) * s2
    W3p = s2[:, None] * W3.astype(f64)
    b3p = b3.astype(f64) + t2 @ W3.astype(f64)
    W4 = W4.astype(f64); b4 = b4.astype(f64)

    def g(x):
        h = np.maximum(x[:, None] * w1 + b1, 0.0)
        h = np.maximum(h @ W2p + b2p, 0.0)
        h = np.maximum(h @ W3p + b3p, 0.0)
        return (h @ W4 + b4)[:, 0]

    pad = 0.02 * max(xhi - xlo, 1e-3)
    xs = np.linspace(xlo - pad, xhi + pad, 4097)
    gx = g(xs)
    fx = 1.0 / (1.0 + np.exp(-gx))
    # weight the fit of p~g by sigmoid'(g): err through sigmoid ~ w*(p-g)
    w = fx * (1.0 - fx) + 1e-3
    best = None
    for d in range(1, 9):
        V = np.vander(xs, d + 1, increasing=True)
        coef, *_ = np.linalg.lstsq(V * w[:, None], gx * w, rcond=None)
        fit = 1.0 / (1.0 + np.exp(-(V @ coef)))
        err = np.abs(fit - fx).max()
        if best is None or err < best[1]:
            best = (coef, err)
        if err < 1.5e-3:
            break
    return best


def _build_program_tile(coef, rw):
    """Build the SPMD Bacc program (one core's view). Returns finalized nc."""
    import concourse.bacc as bacc
    import concourse.mybir as mybir
    from concourse.tile import TileContext

    dt = mybir.dt
    ALU = mybir.AluOpType
    ACTF = mybir.ActivationFunctionType

    ca = 0.5 * (1.0 - rw)   # scale on the residual part
    d = len(coef) - 1

    nc = bacc.Bacc()
    a = nc.declare_dram_parameter("a", [N, N], dt.float32, isOutput=False)
    # cpack: crw-scaled column mask [P,N] | (1-I) [P,P] | I [P,P] | bias, bf16
    cpack = nc.declare_dram_parameter("cpack", [P, N + 2 * P + NT], dt.bfloat16,
                                      isOutput=False)
    # bpack[:, t] = c0 - BIG*(1 - m[t*P+p]): sigmoid bias with row mask folded
    bpack = nc.declare_dram_parameter("bpack", [P, NT], dt.float32, isOutput=False)
    out = nc.declare_dram_parameter("out", [N, N], dt.float32, isOutput=True)

    with TileContext(nc) as tc:
        with (
            tc.tile_pool(name="sb", bufs=1) as sb,
            tc.tile_pool(name="ps", bufs=1, space="PSUM") as ps,
        ):
            a_sb = sb.tile([P, NT * N], dt.float32, name="a_sb")
            abf = sb.tile([P, NT * N], dt.bfloat16, name="abf")
            F = sb.tile([P, NT * N], dt.bfloat16, name="F")
            Z = sb.tile([P, NT * N], dt.bfloat16, name="Z")
            osb = sb.tile([P, NT * N], dt.float32, name="osb")
            cpack_sb = sb.tile([P, N + 2 * P + NT], dt.bfloat16, name="cpack_sb")
            bpack_sb = sb.tile([P, NT], dt.float32, name="bpack_sb")
            mcol_sb = cpack_sb[:, 0:N]
            diag_sb = cpack_sb[:, N:N + P]
            id_sb = cpack_sb[:, N + P:N + 2 * P]
            pt = ps.tile([P, NT * N], dt.bfloat16, name="pt")
            warm = sb.tile([P, 1], dt.float32, name="warm")

            a3 = a.rearrange("(t p) j -> p t j", p=P)
            asb3 = a_sb[:].rearrange("p (t j) -> p t j", j=N)
            nc.sync.dma_start(out=bpack_sb[:], in_=bpack[:])
            nc.sync.dma_start(out=asb3[:, 0:1, :], in_=a3[:, 0:1, :])
            nc.sync.dma_start(out=cpack_sb[:], in_=cpack[:])
            nc.sync.dma_start(out=asb3[:, 1:2, :], in_=a3[:, 1:2, :])
            nc.sync.dma_start(out=asb3[:, 2:4, :], in_=a3[:, 2:4, :])

            # warm the sigmoid table while input DMAs run (scratch tile)
            nc.vector.memset(warm[:], 0.0)
            nc.scalar.activation(warm[:], warm[:], ACTF.Sigmoid)

            for t in range(NT):
                s = slice(t * N, (t + 1) * N)
                bias_t = bpack_sb[:, t:t + 1]

                if d == 1:
                    # F = sigmoid(c1*a + c0 - BIG*(1-m_row)) == m_row*sigmoid(p(a))
                    nc.scalar.activation(F[:, s], a_sb[:, s], ACTF.Sigmoid,
                                         bias=bias_t, scale=float(coef[1]))
                    if t % 2 == 0:
                        # abf = ca * A in bf16 (free scale on the ACT copy)
                        h = slice(t * N, (t + 2) * N)
                        nc.scalar.mul(abf[:, h], a_sb[:, h], ca)
                else:
                    # Horner-style chain: acc = c_d*x + c_{d-1};
                    # acc = (acc + c_k)*x ...; F = sigmoid(acc + bias)
                    nc.vector.tensor_scalar(F[:, s], a_sb[:, s], float(coef[d]),
                                            float(coef[d - 1]), ALU.mult, ALU.add)
                    for k in range(d - 2, 0, -1):
                        nc.vector.scalar_tensor_tensor(
                            F[:, s], F[:, s], float(coef[k]), a_sb[:, s],
                            ALU.add, ALU.mult)
                    nc.scalar.activation(F[:, s], F[:, s], ACTF.Sigmoid,
                                         bias=bias_t, scale=1.0)

                # column mask (carries crw), then residual Z = ca*A + F
                nc.vector.tensor_tensor(F[:, s], F[:, s], mcol_sb, ALU.mult)
                if d == 1:
                    nc.vector.tensor_tensor(Z[:, s], abf[:, s], F[:, s], ALU.add)
                else:
                    nc.vector.scalar_tensor_tensor(Z[:, s], a_sb[:, s], ca, F[:, s],
                                                   ALU.mult, ALU.add)
                # zero this row tile's diagonal block in one strided op
                db = slice(t * N + t * P, t * N + (t + 1) * P)
                nc.vector.tensor_tensor(Z[:, db], Z[:, db], diag_sb, ALU.mult)

                # transposes of this column of blocks into each psum row tile
                for r in range(NT):
                    blk = slice(t * N + r * P, t * N + (r + 1) * P)
                    nc.tensor.transpose(pt[:, r * N + t * P:r * N + (t + 1) * P],
                                        Z[:, blk], id_sb)

            for r in range(NT):
                s = slice(r * N, (r + 1) * N)
                nc.vector.tensor_tensor(osb[:, s], Z[:, s], pt[:, s], ALU.add)
                nc.sync.dma_start(out=out[r * P:(r + 1) * P, :], in_=osb[:, s])

    nc.finalize()
    return nc


def _build_program_raw(coef, rw):
    """Raw bacc program (manual semaphores) for the linear d==1 case.

    Engine split: SP issues all DMAs; ACT does the 4 sigmoids (row mask in
    the per-partition bias); GpSimd does the ca*A bf16 casts; DVE does
    column-mask TT, residual add TT, diag zero TT and the 4 symmetrize
    adds from PSUM; PE does the 16 128x128 block transposes.
    """
    from contextlib import ExitStack

    import concourse.bacc as bacc
    import concourse.mybir as mybir

    dt = mybir.dt
    ALU = mybir.AluOpType
    ACTF = mybir.ActivationFunctionType

    ca = 0.5 * (1.0 - rw)
    # tile processing order = expected input-arrival order; a0/a1 share one
    # DMA+semaphore (index 0), a2 -> sem 2, a3 -> sem 3
    ORDER = [0, 1, 3, 2]
    SEM_OF = {0: 0, 1: 0, 2: 2, 3: 3}

    nc = bacc.Bacc()
    a = nc.declare_dram_parameter("a", [N, N], dt.float32, isOutput=False)
    cpack = nc.declare_dram_parameter("cpack", [P, N + 2 * P + NT], dt.bfloat16,
                                      isOutput=False)
    out = nc.declare_dram_parameter("out", [N, N], dt.float32, isOutput=True)

    es = ExitStack()
    a_sb = es.enter_context(nc.sbuf_tensor("a_sb", [P, NT * N], dt.float32))
    abf = es.enter_context(nc.sbuf_tensor("abf", [P, NT * N], dt.bfloat16))
    F = es.enter_context(nc.sbuf_tensor("F", [P, NT * N], dt.bfloat16))
    Z = es.enter_context(nc.sbuf_tensor("Z", [P, NT * N], dt.bfloat16))
    osb = es.enter_context(nc.sbuf_tensor("osb", [P, NT * N], dt.float32))
    cpack_sb = es.enter_context(nc.sbuf_tensor("cpack_sb", [P, N + 2 * P + NT], dt.bfloat16))
    warm_sb = es.enter_context(nc.sbuf_tensor("warm_sb", [P, 1], dt.float32))
    # single PSUM tensor [P, 2048] bf16 = 2 banks: rows 0,1 in bank0 and
    # rows 3,2 in bank1, so the paired final adds never read a bank that
    # PE transposes still write (same-bank PE-W + DVE-R is fatal)
    pt = es.enter_context(nc.psum_tensor("pt", [P, NT * N], dt.bfloat16))

    s_cp = es.enter_context(nc.semaphore("s_cp"))
    s_a = [es.enter_context(nc.semaphore(f"s_a{t}")) for t in range(NT)]
    s_sig = es.enter_context(nc.semaphore("s_sig"))
    s_z = es.enter_context(nc.semaphore("s_z"))
    s_zd = es.enter_context(nc.semaphore("s_zd"))
    s_tr = es.enter_context(nc.semaphore("s_tr"))
    s_fin = es.enter_context(nc.semaphore("s_fin"))
    s_out = es.enter_context(nc.semaphore("s_out"))

    block = es.enter_context(nc.Block())

    mcol_sb = cpack_sb[:, 0:N]
    diag_sb = cpack_sb[:, N:N + P]
    id_sb = cpack_sb[:, N + P:N + 2 * P]
    bias_sb = cpack_sb[:, N + 2 * P:N + 2 * P + NT]
    a3 = a.rearrange("(t p) j -> p t j", p=P)
    asb3 = a_sb[:].rearrange("p (t j) -> p t j", j=N)

    @block.sync
    def _(sync):
        # qSPDynamicHW ring: a0+a1 then a2; outs for rows 0 and 2
        sync.dma_start(out=asb3[:, 0:2, :], in_=a3[:, 0:2, :]).then_inc(s_a[0], 16)
        sync.dma_start(out=asb3[:, 2:3, :], in_=a3[:, 2:3, :]).then_inc(s_a[2], 16)
        for r, gate in ((0, 1), (2, 2)):
            sync.wait_ge(s_fin, gate)
            sync.dma_start(out=out[r * P:(r + 1) * P, :],
                           in_=osb[:, r * N:(r + 1) * N]).then_inc(s_out, 16)

    @block.scalar
    def _(scalar):
        # first ACTIVATE triggers the sigmoid table load; do it on scratch
        scalar.activation(warm_sb[:], warm_sb[:], ACTF.Sigmoid, bias=0.0)
        # qActDynamicHW ring: cpack, a3 (overlaps the table load)
        scalar.dma_start(out=cpack_sb[:], in_=cpack[:]).then_inc(s_cp, 16)
        scalar.dma_start(out=asb3[:, 3:4, :], in_=a3[:, 3:4, :]).then_inc(s_a[3], 16)
        scalar.wait_ge(s_cp, 16)
        for t in ORDER:
            scalar.wait_ge(s_a[SEM_OF[t]], 16)
            scalar.activation(F[:, t * N:(t + 1) * N], a_sb[:, t * N:(t + 1) * N],
                              ACTF.Sigmoid, bias=bias_sb[:, t:t + 1],
                              scale=float(coef[1])).then_inc(s_sig, 1)
        for r, gate in ((1, 1), (3, 2)):
            scalar.wait_ge(s_fin, gate)
            scalar.dma_start(out=out[r * P:(r + 1) * P, :],
                             in_=osb[:, r * N:(r + 1) * N]).then_inc(s_out, 16)

    import dataclasses as _dc
    mcol2 = _dc.replace(mcol_sb, ap=type(mcol_sb.ap)(
        [[mcol_sb.ap[0][0], P], [0, 2], [1, N]]))

    @block.vector
    def _(vector):
        # paired tiles 0+1 (one input DMA/sem), then 3, then 2
        s01 = slice(0, 2 * N)
        vector.wait_ge(s_sig, 2)
        vector.tensor_tensor(F[:, s01].rearrange("p (t j) -> p t j", j=N),
                             F[:, s01].rearrange("p (t j) -> p t j", j=N),
                             mcol2, ALU.mult)
        vector.scalar_tensor_tensor(Z[:, s01], a_sb[:, s01], ca, F[:, s01],
                                    ALU.mult, ALU.add).then_inc(s_z, 1)
        for t in (0, 1):
            db = slice(t * N + t * P, t * N + (t + 1) * P)
            vector.tensor_tensor(Z[:, db], Z[:, db], diag_sb,
                                 ALU.mult).then_inc(s_zd, 1)
        for k, t in ((2, 3), (3, 2)):
            s = slice(t * N, (t + 1) * N)
            vector.wait_ge(s_sig, k + 1)
            vector.tensor_tensor(F[:, s], F[:, s], mcol_sb, ALU.mult)
            vector.scalar_tensor_tensor(Z[:, s], a_sb[:, s], ca, F[:, s],
                                        ALU.mult, ALU.add).then_inc(s_z, 1)
            db = slice(t * N + t * P, t * N + (t + 1) * P)
            vector.tensor_tensor(Z[:, db], Z[:, db], diag_sb,
                                 ALU.mult).then_inc(s_zd, 1)
        # paired symmetrize adds: rows 0+1 (bank 0), then rows 2+3 (bank 1)
        vector.wait_ge(s_tr, 14)
        vector.tensor_tensor(osb[:, 0:2 * N], Z[:, 0:2 * N],
                             pt[:, 0:2 * N], ALU.add).then_inc(s_fin, 1)
        vector.wait_ge(s_tr, 16)
        vector.tensor_tensor(osb[:, 2 * N:4 * N], Z[:, 2 * N:4 * N],
                             pt[:, 2 * N:4 * N], ALU.add).then_inc(s_fin, 1)

    @block.tensor
    def _(tensor):
        # s_z counts: 1 after tiles 0+1, 2 after tile 3, 3 after tile 2;
        # s_zd counts diag zeroes in processing order 0,1,3,2
        ZGATE = {0: 1, 1: 1, 3: 2, 2: 3}
        DGATE = {0: 1, 1: 2, 3: 3, 2: 4}
        for t in ORDER:
            tensor.wait_ge(s_z, ZGATE[t])
            for r in ORDER:
                if r == t:
                    continue
                blk = slice(t * N + r * P, t * N + (r + 1) * P)
                tensor.transpose(pt[:, r * N + t * P:r * N + (t + 1) * P],
                                 Z[:, blk], id_sb).then_inc(s_tr, 1)
            tensor.wait_ge(s_zd, DGATE[t])
            blk = slice(t * N + t * P, t * N + (t + 1) * P)
            tensor.transpose(pt[:, t * N + t * P:t * N + (t + 1) * P],
                             Z[:, blk], id_sb).then_inc(s_tr, 1)

    es.close()
    nc.finalize()
    return nc


_CACHE = {}


def _make_in_maps(sim, masks, coef, rw):
    bf16 = ml_dtypes.bfloat16
    crw = 0.5 * rw
    # logit offset that guarantees sigmoid ~ 0 on masked rows, whatever
    # the fitted polynomial's range is on the observed inputs
    xs = np.linspace(float(sim.min()), float(sim.max()), 257)
    pmax = float(np.abs(np.polyval(coef[::-1], xs)).max())
    big = BIG + pmax
    mf = masks.astype(np.float32)
    ident = np.eye(P, dtype=np.float32)
    diagm = 1.0 - ident
    in_maps = []
    for b in range(B):
        mcol = np.broadcast_to(crw * mf[b], (P, N))
        bias = float(coef[0]) - big * (1.0 - mf[b].reshape(NT, P).T)
        cpack = np.concatenate([mcol, diagm, ident, bias], axis=1).astype(bf16)
        in_maps.append(dict(a=sim[b], cpack=cpack,
                            bpack=bias.astype(np.float32).copy()))
    return in_maps


def kernel(similarity_matrix, node_masks, W1, b1, g1, be1, m1, v1,
           W2, b2, g2, be2, m2, v2, W3, b3, W4, b4, residual_weight):
    from concourse.bass_utils import run_bass_kernel_spmd

    sim = np.asarray(similarity_matrix, dtype=np.float32)
    masks = np.asarray(node_masks)
    assert sim.shape == (B, N, N), sim.shape
    rw = float(np.asarray(residual_weight))

    coef, fit_err = _fit_scalar_fn(
        np.asarray(W1)[0], np.asarray(b1), np.asarray(W2), np.asarray(b2),
        np.asarray(g1), np.asarray(be1), np.asarray(m1), np.asarray(v1),
        np.asarray(g2), np.asarray(be2), np.asarray(m2), np.asarray(v2),
        np.asarray(W3), np.asarray(b3), np.asarray(W4), np.asarray(b4),
        float(sim.min()), float(sim.max()))

    key = (tuple(np.round(coef, 12)), round(rw, 12))
    if key not in _CACHE:
        if len(coef) == 2:
            _CACHE[key] = _build_program_raw(coef, rw)
        else:
            _CACHE[key] = _build_program_tile(coef, rw)
    nc = _CACHE[key]

    in_maps = _make_in_maps(sim, masks, coef, rw)
    res = run_bass_kernel_spmd(nc, in_maps, core_ids=list(range(B)))
    out = np.stack([res.results[b]["out"] for b in range(B)], axis=0)
    return out.astype(np.float32)


# revision 26
# speedup vs baseline: 1.0343x; 1.0343x over previous
"""Trainium2 Bass kernel for nn_A100SimilarityCorrector.

Full inputs in, full output out. Data-parallel over the batch: 8 batch
elements -> 8 NeuronCores, one [512,512] similarity slice per core.

Key structure: in eval mode the 4-layer MLP acts independently on each
scalar similarity x, so it collapses to a 1-D function f(x) = sigmoid(g(x))
with g piecewise-linear. BatchNorms fold exactly into the weights; g is
fit on the host with a low-degree polynomial p (exact linear for the
default zero-bias weights), and the device evaluates
    out = (1-I) o (Z + Z^T),  Z = 0.5*rw*(m m^T) o sigmoid(p(A)) + 0.5*(1-rw)*A
with ScalarE (sigmoid; the row mask rides the per-partition bias as a
-50 logit offset), VectorE (column mask / residual / symmetrize adds),
TensorE (16 x 128x128 block transposes into PSUM) and DMA in/out.
"""
import dataclasses
import sys

sys.path.insert(0, "/opt/trn_rl_repo")

import numpy as np
import ml_dtypes

EPS = 1e-5
B, N, P = 8, 512, 128
NT = N // P  # 4 row tiles per core
BIG = 50.0   # logit offset that drives sigmoid to 0 for masked rows


def _fit_scalar_fn(w1, b1, W2, b2, g1, be1, m1, v1, g2, be2, m2, v2,
                   W3, b3, W4, b4, xlo, xhi):
    """Fold BN into weights, then least-squares fit poly p with
    sigmoid(p(x)) ~ f(x) on [xlo, xhi]. Returns (coeffs lowest-first, max_err)."""
    f64 = np.float64
    w1 = w1.astype(f64); b1 = b1.astype(f64)
    s1 = g1.astype(f64) / np.sqrt(v1.astype(f64) + EPS)
    t1 = be1.astype(f64) - m1.astype(f64) * s1
    W2p = s1[:, None] * W2.astype(f64)
    b2p = b2.astype(f64) + t1 @ W2.astype(f64)
    s2 = g2.astype(f64) / np.sqrt(v2.astype(f64) + EPS)
    t2 = be2.astype(f64) - m2.astype(f64) * s2
    W3p = s2[:, None] * W3.astype(f64)
    b3p = b3.astype(f64) + t2 @ W3.astype(f64)
    W4 = W4.astype(f64); b4 = b4.astype(f64)

    def g(x):
        h = np.maximum(x[:, None] * w1 + b1, 0.0)
        h = np.maximum(h @ W2p + b2p, 0.0)
        h = np.maximum(h @ W3p + b3p, 0.0)
        return (h @ W4 + b4)[:, 0]

    pad = 0.02 * max(xhi - xlo, 1e-3)
    xs = np.linspace(xlo - pad, xhi + pad, 4097)
    gx = g(xs)
    fx = 1.0 / (1.0 + np.exp(-gx))
    # weight the fit of p~g by sigmoid'(g): err through sigmoid ~ w*(p-g)
    w = fx * (1.0 - fx) + 1e-3
    best = None
    for d in range(1, 9):
        V = np.vander(xs, d + 1, increasing=True)
        coef, *_ = np.linalg.lstsq(V * w[:, None], gx * w, rcond=None)
        fit = 1.0 / (1.0 + np.exp(-(V @ coef)))
        err = np.abs(fit - fx).max()
        if best is None or err < best[1]:
            best = (coef, err)
        if err < 1.5e-3:
            break
    return best


def _build_program_tile(coef, rw):
    """Build the SPMD Bacc program (one core's view). Returns finalized nc."""
    import concourse.bacc as bacc
    import concourse.mybir as mybir
    from concourse.tile import TileContext

    dt = mybir.dt
    ALU = mybir.AluOpType
    ACTF = mybir.ActivationFunctionType

    ca = 0.5 * (1.0 - rw)   # scale on the residual part
    d = len(coef) - 1

    nc = bacc.Bacc()
    a = nc.declare_dram_parameter("a", [N, N], dt.float32, isOutput=False)
    # cpack: crw-scaled column mask [P,N] | (1-I) [P,P] | I [P,P] | bias, bf16
    cpack = nc.declare_dram_parameter("cpack", [P, N + 2 * P + NT], dt.bfloat16,
                                      isOutput=False)
    # bpack[:, t] = c0 - BIG*(1 - m[t*P+p]): sigmoid bias with row mask folded
    bpack = nc.declare_dram_parameter("bpack", [P, NT], dt.float32, isOutput=False)
    out = nc.declare_dram_parameter("out", [N, N], dt.float32, isOutput=True)

    with TileContext(nc) as tc:
        with (
            tc.tile_pool(name="sb", bufs=1) as sb,
            tc.tile_pool(name="ps", bufs=1, space="PSUM") as ps,
        ):
            a_sb = sb.tile([P, NT * N], dt.float32, name="a_sb")
            abf = sb.tile([P, NT * N], dt.bfloat16, name="abf")
            F = sb.tile([P, NT * N], dt.bfloat16, name="F")
            Z = sb.tile([P, NT * N], dt.bfloat16, name="Z")
            osb = sb.tile([P, NT * N], dt.float32, name="osb")
            cpack_sb = sb.tile([P, N + 2 * P + NT], dt.bfloat16, name="cpack_sb")
            bpack_sb = sb.tile([P, NT], dt.float32, name="bpack_sb")
            mcol_sb = cpack_sb[:, 0:N]
            diag_sb = cpack_sb[:, N:N + P]
            id_sb = cpack_sb[:, N + P:N + 2 * P]
            pt = ps.tile([P, NT * N], dt.bfloat16, name="pt")
            warm = sb.tile([P, 1], dt.float32, name="warm")

            a3 = a.rearrange("(t p) j -> p t j", p=P)
            asb3 = a_sb[:].rearrange("p (t j) -> p t j", j=N)
            nc.sync.dma_start(out=bpack_sb[:], in_=bpack[:])
            nc.sync.dma_start(out=asb3[:, 0:1, :], in_=a3[:, 0:1, :])
            nc.sync.dma_start(out=cpack_sb[:], in_=cpack[:])
            nc.sync.dma_start(out=asb3[:, 1:2, :], in_=a3[:, 1:2, :])
            nc.sync.dma_start(out=asb3[:, 2:4, :], in_=a3[:, 2:4, :])

            # warm the sigmoid table while input DMAs run (scratch tile)
            nc.vector.memset(warm[:], 0.0)
            nc.scalar.activation(warm[:], warm[:], ACTF.Sigmoid)

            for t in range(NT):
                s = slice(t * N, (t + 1) * N)
                bias_t = bpack_sb[:, t:t + 1]

                if d == 1:
                    # F = sigmoid(c1*a + c0 - BIG*(1-m_row)) == m_row*sigmoid(p(a))
                    nc.scalar.activation(F[:, s], a_sb[:, s], ACTF.Sigmoid,
                                         bias=bias_t, scale=float(coef[1]))
                    if t % 2 == 0:
                        # abf = ca * A in bf16 (free scale on the ACT copy)
                        h = slice(t * N, (t + 2) * N)
                        nc.scalar.mul(abf[:, h], a_sb[:, h], ca)
                else:
                    # Horner-style chain: acc = c_d*x + c_{d-1};
                    # acc = (acc + c_k)*x ...; F = sigmoid(acc + bias)
                    nc.vector.tensor_scalar(F[:, s], a_sb[:, s], float(coef[d]),
                                            float(coef[d - 1]), ALU.mult, ALU.add)
                    for k in range(d - 2, 0, -1):
                        nc.vector.scalar_tensor_tensor(
                            F[:, s], F[:, s], float(coef[k]), a_sb[:, s],
                            ALU.add, ALU.mult)
                    nc.scalar.activation(F[:, s], F[:, s], ACTF.Sigmoid,
                                         bias=bias_t, scale=1.0)

                # column mask (carries crw), then residual Z = ca*A + F
                nc.vector.tensor_tensor(F[:, s], F[:, s], mcol_sb, ALU.mult)
                if d == 1:
                    nc.vector.tensor_tensor(Z[:, s], abf[:, s], F[:, s], ALU.add)
                else:
                    nc.vector.scalar_tensor_tensor(Z[:, s], a_sb[:, s], ca, F[:, s],
                                                   ALU.mult, ALU.add)
                # zero this row tile's diagonal block in one strided op
                db = slice(t * N + t * P, t * N + (t + 1) * P)
                nc.vector.tensor_tensor(Z[:, db], Z[:, db], diag_sb, ALU.mult)

                # transposes of this column of blocks into each psum row tile
                for r in range(NT):
                    blk = slice(t * N + r * P, t * N + (r + 1) * P)
                    nc.tensor.transpose(pt[:, r * N + t * P:r * N + (t + 1) * P],
                                        Z[:, blk], id_sb)

            for r in range(NT):
                s = slice(r * N, (r + 1) * N)
                nc.vector.tensor_tensor(osb[:, s], Z[:, s], pt[:, s], ALU.add)
                nc.sync.dma_start(out=out[r * P:(r + 1) * P, :], in_=osb[:, s])

    nc.finalize()
    return nc


def _build_program_raw(coef, rw):
    """Raw bacc program (manual semaphores) for the linear d==1 case.

    Engine split: SP issues all DMAs; ACT does the 4 sigmoids (row mask in
    the per-partition bias); GpSimd does the ca*A bf16 casts; DVE does
    column-mask TT, residual add TT, diag zero TT and the 4 symmetrize
    adds from PSUM; PE does the 16 128x128 block transposes.
    """
    from contextlib import ExitStack

    import concourse.bacc as bacc
    import concourse.mybir as mybir

    dt = mybir.dt
    ALU = mybir.AluOpType
    ACTF = mybir.ActivationFunctionType

    ca = 0.5 * (1.0 - rw)
    # tile processing order = expected input-arrival order; a0/a1 share one
    # DMA+semaphore (index 0), a2 -> sem 2, a3 -> sem 3
    ORDER = [0, 1, 3, 2]
    SEM_OF = {0: 0, 1: 0, 2: 2, 3: 3}

    nc = bacc.Bacc()
    a = nc.declare_dram_parameter("a", [N, N], dt.float32, isOutput=False)
    cpack = nc.declare_dram_parameter("cpack", [P, N + 2 * P + NT], dt.bfloat16,
                                      isOutput=False)
    out = nc.declare_dram_parameter("out", [N, N], dt.float32, isOutput=True)

    es = ExitStack()
    a_sb = es.enter_context(nc.sbuf_tensor("a_sb", [P, NT * N], dt.float32))
    abf = es.enter_context(nc.sbuf_tensor("abf", [P, NT * N], dt.bfloat16))
    F = es.enter_context(nc.sbuf_tensor("F", [P, NT * N], dt.bfloat16))
    Z = es.enter_context(nc.sbuf_tensor("Z", [P, NT * N], dt.bfloat16))
    osb = es.enter_context(nc.sbuf_tensor("osb", [P, NT * N], dt.float32))
    cpack_sb = es.enter_context(nc.sbuf_tensor("cpack_sb", [P, N + 2 * P + NT], dt.bfloat16))
    warm_sb = es.enter_context(nc.sbuf_tensor("warm_sb", [P, 1], dt.float32))
    # one PSUM bank per row tile so DVE reads never share a bank with
    # in-flight PE transpose writes (same-bank PE-W + DVE-R is fatal)
    pt = [es.enter_context(nc.psum_tensor(f"pt{r}", [P, 1024], dt.bfloat16))
          for r in range(NT)]

    s_cp = es.enter_context(nc.semaphore("s_cp"))
    s_a = [es.enter_context(nc.semaphore(f"s_a{t}")) for t in range(NT)]
    s_sig = es.enter_context(nc.semaphore("s_sig"))
    s_z = es.enter_context(nc.semaphore("s_z"))
    s_zd = es.enter_context(nc.semaphore("s_zd"))
    s_tr = es.enter_context(nc.semaphore("s_tr"))
    s_fin = es.enter_context(nc.semaphore("s_fin"))
    s_out = es.enter_context(nc.semaphore("s_out"))

    block = es.enter_context(nc.Block())

    mcol_sb = cpack_sb[:, 0:N]
    diag_sb = cpack_sb[:, N:N + P]
    id_sb = cpack_sb[:, N + P:N + 2 * P]
    bias_sb = cpack_sb[:, N + 2 * P:N + 2 * P + NT]
    a3 = a.rearrange("(t p) j -> p t j", p=P)
    asb3 = a_sb[:].rearrange("p (t j) -> p t j", j=N)

    @block.sync
    def _(sync):
        # qSPDynamicHW ring: a0+a1 then a2; outs for rows 0 and 2
        sync.dma_start(out=asb3[:, 0:2, :], in_=a3[:, 0:2, :]).then_inc(s_a[0], 16)
        sync.dma_start(out=asb3[:, 2:3, :], in_=a3[:, 2:3, :]).then_inc(s_a[2], 16)
        for r in (0, 2):
            sync.wait_ge(s_fin, ORDER.index(r) + 1)
            sync.dma_start(out=out[r * P:(r + 1) * P, :],
                           in_=osb[:, r * N:(r + 1) * N]).then_inc(s_out, 16)

    @block.scalar
    def _(scalar):
        # first ACTIVATE triggers the sigmoid table load; do it on scratch
        scalar.activation(warm_sb[:], warm_sb[:], ACTF.Sigmoid, bias=0.0)
        # qActDynamicHW ring: cpack, a3 (overlaps the table load)
        scalar.dma_start(out=cpack_sb[:], in_=cpack[:]).then_inc(s_cp, 16)
        scalar.dma_start(out=asb3[:, 3:4, :], in_=a3[:, 3:4, :]).then_inc(s_a[3], 16)
        scalar.wait_ge(s_cp, 16)
        for t in ORDER:
            scalar.wait_ge(s_a[SEM_OF[t]], 16)
            scalar.activation(F[:, t * N:(t + 1) * N], a_sb[:, t * N:(t + 1) * N],
                              ACTF.Sigmoid, bias=bias_sb[:, t:t + 1],
                              scale=float(coef[1])).then_inc(s_sig, 1)
        for r in (1, 3):
            scalar.wait_ge(s_fin, ORDER.index(r) + 1)
            scalar.dma_start(out=out[r * P:(r + 1) * P, :],
                             in_=osb[:, r * N:(r + 1) * N]).then_inc(s_out, 16)

    @block.vector
    def _(vector):
        for k, t in enumerate(ORDER):
            s = slice(t * N, (t + 1) * N)
            vector.wait_ge(s_sig, k + 1)
            vector.tensor_tensor(F[:, s], F[:, s], mcol_sb, ALU.mult)
            vector.scalar_tensor_tensor(Z[:, s], a_sb[:, s], ca, F[:, s],
                                        ALU.mult, ALU.add).then_inc(s_z, 1)
            db = slice(t * N + t * P, t * N + (t + 1) * P)
            vector.tensor_tensor(Z[:, db], Z[:, db], diag_sb,
                                 ALU.mult).then_inc(s_zd, 1)
        for k, r in enumerate(ORDER):
            vector.wait_ge(s_tr, 13 + k)
            vector.tensor_tensor(osb[:, r * N:(r + 1) * N],
                                 Z[:, r * N:(r + 1) * N],
                                 pt[r][:, 0:N], ALU.add).then_inc(s_fin, 1)

    @block.tensor
    def _(tensor):
        for k, t in enumerate(ORDER):
            # off-diagonal blocks only need the residual add (s_z);
            # the (t,t) block also needs the diag zero (s_zd)
            tensor.wait_ge(s_z, k + 1)
            for j, r in enumerate(ORDER):
                if r == t:
                    continue
                blk = slice(t * N + r * P, t * N + (r + 1) * P)
                tensor.transpose(pt[r][:, t * P:(t + 1) * P], Z[:, blk],
                                 id_sb).then_inc(s_tr, 1)
            tensor.wait_ge(s_zd, k + 1)
            blk = slice(t * N + t * P, t * N + (t + 1) * P)
            tensor.transpose(pt[t][:, t * P:(t + 1) * P], Z[:, blk],
                             id_sb).then_inc(s_tr, 1)

    es.close()
    nc.finalize()
    return nc


_CACHE = {}


def _make_in_maps(sim, masks, coef, rw):
    bf16 = ml_dtypes.bfloat16
    crw = 0.5 * rw
    # logit offset that guarantees sigmoid ~ 0 on masked rows, whatever
    # the fitted polynomial's range is on the observed inputs
    xs = np.linspace(float(sim.min()), float(sim.max()), 257)
    pmax = float(np.abs(np.polyval(coef[::-1], xs)).max())
    big = BIG + pmax
    mf = masks.astype(np.float32)
    ident = np.eye(P, dtype=np.float32)
    diagm = 1.0 - ident
    in_maps = []
    for b in range(B):
        mcol = np.broadcast_to(crw * mf[b], (P, N))
        bias = float(coef[0]) - big * (1.0 - mf[b].reshape(NT, P).T)
        cpack = np.concatenate([mcol, diagm, ident, bias], axis=1).astype(bf16)
        in_maps.append(dict(a=sim[b], cpack=cpack,
                            bpack=bias.astype(np.float32).copy()))
    return in_maps


def kernel(similarity_matrix, node_masks, W1, b1, g1, be1, m1, v1,
           W2, b2, g2, be2, m2, v2, W3, b3, W4, b4, residual_weight):
    from concourse.bass_utils import run_bass_kernel_spmd

    sim = np.asarray(similarity_matrix, dtype=np.float32)
    masks = np.asarray(node_masks)
    assert sim.shape == (B, N, N), sim.shape
    rw = float(np.asarray(residual_weight))

    coef, fit_err = _fit_scalar_fn(
        np.asarray(W1)[0], np.asarray(b1), np.asarray(W2), np.asarray(b2),
        np.asarray(g1), np.asarray(be1), np.asarray(m1), np.asarray(v1),
        np.asarray(g2), np.asarray(be2), np.asarray(m2), np.asarray(v2),
        np.asarray(W3), np.asarray(b3), np.asarray(W4), np.asarray(b4),
        float(sim.min()), float(sim.max()))

    key = (tuple(np.round(coef, 12)), round(rw, 12))
    if key not in _CACHE:
        if len(coef) == 2:
            _CACHE[key] = _build_program_raw(coef, rw)
        else:
            _CACHE[key] = _build_program_tile(coef, rw)
    nc = _CACHE[key]

    in_maps = _make_in_maps(sim, masks, coef, rw)
    res = run_bass_kernel_spmd(nc, in_maps, core_ids=list(range(B)))
    out = np.stack([res.results[b]["out"] for b in range(B)], axis=0)
    return out.astype(np.float32)
